# revision 4
# baseline (speedup 1.0000x reference)
"""Conditional per-sample 64x64 matmul (MoE-style routing), Trainium2 Bass kernel.

out[b, d, t] = sum_c x[b, c, t] * weights[cond_ids[b], c, d]

Strategy:
  - Host gathers the per-sample weight [B, Cin, Cout] (tiny) and packs
    adjacent sample pairs into block-diagonal [128, 128] stationary
    matrices so each matmul uses all 128 PE rows / SBUF partitions.
  - Data-parallel across 8 NeuronCores over the batch: 16 samples
    (= 8 pairs) per core.
  - Per pair: x slice is a [128, 8192] f32 view (2 samples x 64 chans).
    Stream T in chunks of 2048: DMA 1MiB in -> 4 matmuls (K=128, N=512)
    into a 4-bank PSUM tile -> one DVE copy to SBUF -> DMA 1MiB out.
  - Executed through the same bass_exec/PJRT path run_bass_kernel_spmd
    uses under axon, but with the jitted executable cached so repeated
    kernel() calls don't re-trace/re-compile.
"""

import numpy as np

import jax
import jax.numpy as jnp
from jax.experimental.shard_map import shard_map
from jax.sharding import Mesh, NamedSharding, PartitionSpec

import concourse.bacc as bacc
import concourse.bass as bass
import concourse.mybir as mybir
import concourse.tile as tile
from concourse.bass2jax import (
    _bass_exec_p,
    install_neuronx_cc_hook,
    partition_id_tensor,
)

B = 128
CIN = 64
COUT = 64
T = 8192
NCORES = 8
PAIRS = B // 2                   # 64 sample pairs
PPC = PAIRS // NCORES            # 8 pairs per core
CHUNK = 2048                     # T chunk per DMA (1 MiB tiles)
MMFREE = 512                     # matmul free dim (one PSUM bank, fp32)

_NC_CACHE = {}
_RUNNER_CACHE = {}
_ZEROS = None


def _build_nc(reps: int = 1):
    f32 = mybir.dt.float32
    nc = bacc.Bacc("TRN2", target_bir_lowering=False, debug=False)

    x_d = nc.dram_tensor("x", [PPC, 128, T], f32, kind="ExternalInput").ap()
    w_d = nc.dram_tensor("wp", [PPC, 128, 128], f32, kind="ExternalInput").ap()
    o_d = nc.dram_tensor("out", [PPC, 128, T], f32, kind="ExternalOutput").ap()

    with tile.TileContext(nc) as tc:
        with (
            tc.tile_pool(name="wpool", bufs=2) as wpool,
            tc.tile_pool(name="xpool", bufs=3) as xpool,
            tc.tile_pool(name="opool", bufs=3) as opool,
            tc.tile_pool(name="pspool", bufs=2, space=bass.MemorySpace.PSUM) as pspool,
        ):
            for _ in range(reps):
                for p in range(PPC):
                    w_t = wpool.tile([128, 128], f32)
                    nc.sync.dma_start(out=w_t[:], in_=w_d[p])
                    for j in range(T // CHUNK):
                        x_t = xpool.tile([128, CHUNK], f32)
                        nc.sync.dma_start(
                            out=x_t[:], in_=x_d[p, :, j * CHUNK : (j + 1) * CHUNK]
                        )
                        ps_t = pspool.tile([128, CHUNK], f32)
                        for k in range(CHUNK // MMFREE):
                            nc.tensor.matmul(
                                ps_t[:, k * MMFREE : (k + 1) * MMFREE],
                                w_t[:],
                                x_t[:, k * MMFREE : (k + 1) * MMFREE],
                            )
                        o_t = opool.tile([128, CHUNK], f32)
                        nc.vector.tensor_copy(o_t[:], ps_t[:])
                        nc.sync.dma_start(
                            out=o_d[p, :, j * CHUNK : (j + 1) * CHUNK], in_=o_t[:]
                        )
    nc.compile()
    return nc


def _get_nc(reps: int = 1):
    if reps not in _NC_CACHE:
        _NC_CACHE[reps] = _build_nc(reps)
    return _NC_CACHE[reps]


def make_runner(reps: int = 1):
    """Jitted sharded executable for the bass program; cached across calls.

    Takes global arrays x_pairs [PAIRS,128,T], wp [PAIRS,128,128],
    zeros [PAIRS,128,T]; returns global out [PAIRS,128,T].
    Mirrors concourse.bass2jax.run_bass_via_pjrt's multi-core path
    (operands must be jit parameters, in order, for neuronx_cc_hook).
    """
    if reps in _RUNNER_CACHE:
        return _RUNNER_CACHE[reps]
    install_neuronx_cc_hook()
    nc = _get_nc(reps)
    out_aval = jax.core.ShapedArray((PPC, 128, T), np.float32)

    def _body(x, wp, z):
        outs = _bass_exec_p.bind(
            x,
            wp,
            z,
            partition_id_tensor(),
            out_avals=(out_aval,),
            in_names=("x", "wp", "out", "partition_id"),
            out_names=("out",),
            lowering_input_output_aliases=(),
            sim_require_finite=True,
            sim_require_nnan=True,
            nc=nc,
        )
        return outs[0]

    devices = jax.devices()[:NCORES]
    mesh = Mesh(np.asarray(devices), ("core",))
    spec = PartitionSpec("core")
    fn = jax.jit(
        shard_map(
            _body,
            mesh=mesh,
            in_specs=(spec, spec, spec),
            out_specs=spec,
            check_rep=False,
        )
    )
    _RUNNER_CACHE[reps] = (fn, mesh)
    return fn, mesh


def _get_zeros(mesh):
    # Device-resident, sharded zero buffer for the NEFF "out" input slot.
    # The kernel overwrites every element, so contents are irrelevant and
    # the buffer can be reused across calls (never donated).
    global _ZEROS
    if _ZEROS is None:
        sharding = NamedSharding(mesh, PartitionSpec("core"))
        _ZEROS = jax.jit(
            lambda: jnp.zeros((PAIRS, 128, T), jnp.float32),
            out_shardings=sharding,
        )()
    return _ZEROS


def kernel(x: np.ndarray, weights: np.ndarray, cond_ids: np.ndarray) -> np.ndarray:
    x = np.ascontiguousarray(np.asarray(x, dtype=np.float32))
    weights = np.asarray(weights, dtype=np.float32)
    cond_ids = np.asarray(cond_ids, dtype=np.int32)

    # Host-side routing: gather per-sample weights, pack sample pairs into
    # block-diagonal [128, 128] stationary matrices.
    w_full = weights[cond_ids]                      # [B, CIN, COUT]
    wp = np.zeros((PAIRS, 2 * CIN, 2 * COUT), dtype=np.float32)
    wp[:, :CIN, :COUT] = w_full[0::2]
    wp[:, CIN:, COUT:] = w_full[1::2]

    x_pairs = x.reshape(PAIRS, 2 * CIN, T)          # zero-copy view

    fn, mesh = make_runner(reps=1)
    out = fn(x_pairs, wp, _get_zeros(mesh))
    return np.asarray(out).reshape(B, COUT, T)


# revision 12
# speedup vs baseline: 1.1283x; 1.1283x over previous
"""Conditional per-sample 64x64 matmul (MoE-style routing), Trainium2 Bass kernel.

out[b, d, t] = sum_c x[b, c, t] * weights[cond_ids[b], c, d]

Strategy:
  - Host gathers the per-sample weight [B, Cin, Cout] (tiny) and packs
    adjacent sample pairs into block-diagonal [128, 128] stationary
    matrices so each matmul uses all 128 PE rows / SBUF partitions.
  - Data-parallel across 8 NeuronCores over the batch: 16 samples
    (= 8 pairs) per core.
  - Per pair: x slice is a [128, 8192] f32 view (2 samples x 64 chans).
    Stream T in chunks of 2048: DMA 1MiB in -> 4 matmuls (K=128, N=512)
    into a 4-bank PSUM tile -> one DVE copy to SBUF -> DMA 1MiB out.
  - Executed through the same bass_exec/PJRT path run_bass_kernel_spmd
    uses under axon, but with the jitted executable cached so repeated
    kernel() calls don't re-trace/re-compile.
"""

import numpy as np

import jax
import jax.numpy as jnp
from jax.experimental.shard_map import shard_map
from jax.sharding import Mesh, NamedSharding, PartitionSpec

import concourse.bacc as bacc
import concourse.bass as bass
import concourse.mybir as mybir
import concourse.tile as tile
from concourse.bass2jax import (
    _bass_exec_p,
    install_neuronx_cc_hook,
    partition_id_tensor,
)

B = 128
CIN = 64
COUT = 64
T = 8192
NCORES = 8
PAIRS = B // 2                   # 64 sample pairs
PPC = PAIRS // NCORES            # 8 pairs per core
CHUNK = 4096                     # T chunk per DMA (2 MiB tiles)
MMFREE = 512                     # matmul free dim (one PSUM bank, fp32)

_NC_CACHE = {}
_RUNNER_CACHE = {}
_ZEROS = None


def _build_nc(
    reps: int = 1,
    chunk: int = CHUNK,
    xbufs: int = 3,
    obufs: int = 3,
    load_eng: str = "sync",
    store_eng: str = "sync",
    compute: bool = True,
    pschunk: int = 2048,
    copy_alt: bool = False,
):
    f32 = mybir.dt.float32
    nc = bacc.Bacc("TRN2", target_bir_lowering=False, debug=False)

    x_d = nc.dram_tensor("x", [PPC, 128, T], f32, kind="ExternalInput").ap()
    w_d = nc.dram_tensor("wp", [PPC, 128, 128], f32, kind="ExternalInput").ap()
    o_d = nc.dram_tensor("out", [PPC, 128, T], f32, kind="ExternalOutput").ap()

    ld = getattr(nc, load_eng)
    st = getattr(nc, store_eng)

    with tile.TileContext(nc) as tc:
        with (
            tc.tile_pool(name="wpool", bufs=2) as wpool,
            tc.tile_pool(name="xpool", bufs=xbufs) as xpool,
            tc.tile_pool(name="opool", bufs=obufs) as opool,
            tc.tile_pool(name="pspool", bufs=2, space=bass.MemorySpace.PSUM) as pspool,
        ):
            for _ in range(reps):
                for p in range(PPC):
                    if compute:
                        w_t = wpool.tile([128, 128], f32)
                        ld.dma_start(out=w_t[:], in_=w_d[p])
                    for j in range(T // chunk):
                        x_t = xpool.tile([128, chunk], f32)
                        ld.dma_start(
                            out=x_t[:], in_=x_d[p, :, j * chunk : (j + 1) * chunk]
                        )
                        if compute:
                            o_t = opool.tile([128, chunk], f32)
                            for h in range(chunk // pschunk):
                                ps_t = pspool.tile([128, pschunk], f32)
                                for k in range(pschunk // MMFREE):
                                    c0 = k * MMFREE
                                    nc.tensor.matmul(
                                        ps_t[:, c0 : c0 + MMFREE],
                                        w_t[:],
                                        x_t[:, h * pschunk + c0 : h * pschunk + c0 + MMFREE],
                                    )
                                dst = o_t[:, h * pschunk : (h + 1) * pschunk]
                                if copy_alt and (j * 8 + h) % 2:
                                    nc.scalar.copy(dst, ps_t[:])
                                else:
                                    nc.vector.tensor_copy(dst, ps_t[:])
                            src = o_t
                        else:
                            src = x_t
                        st.dma_start(
                            out=o_d[p, :, j * chunk : (j + 1) * chunk], in_=src[:]
                        )
    nc.compile()
    return nc


def _get_nc(reps: int = 1, **kw):
    key = (reps, tuple(sorted(kw.items())))
    if key not in _NC_CACHE:
        _NC_CACHE[key] = _build_nc(reps, **kw)
    return _NC_CACHE[key]


def make_runner(reps: int = 1, **kw):
    """Jitted sharded executable for the bass program; cached across calls.

    Takes global arrays x_pairs [PAIRS,128,T], wp [PAIRS,128,128],
    zeros [PAIRS,128,T]; returns global out [PAIRS,128,T].
    Mirrors concourse.bass2jax.run_bass_via_pjrt's multi-core path
    (operands must be jit parameters, in order, for neuronx_cc_hook).
    """
    key = (reps, tuple(sorted(kw.items())))
    if key in _RUNNER_CACHE:
        return _RUNNER_CACHE[key]
    install_neuronx_cc_hook()
    nc = _get_nc(reps, **kw)
    out_aval = jax.core.ShapedArray((PPC, 128, T), np.float32)

    def _body(x, wp, z):
        outs = _bass_exec_p.bind(
            x,
            wp,
            z,
            partition_id_tensor(),
            out_avals=(out_aval,),
            in_names=("x", "wp", "out", "partition_id"),
            out_names=("out",),
            lowering_input_output_aliases=(),
            sim_require_finite=True,
            sim_require_nnan=True,
            nc=nc,
        )
        return outs[0]

    devices = jax.devices()[:NCORES]
    mesh = Mesh(np.asarray(devices), ("core",))
    spec = PartitionSpec("core")
    fn = jax.jit(
        shard_map(
            _body,
            mesh=mesh,
            in_specs=(spec, spec, spec),
            out_specs=spec,
            check_rep=False,
        )
    )
    _RUNNER_CACHE[key] = (fn, mesh)
    return fn, mesh


def _get_zeros(mesh):
    # Device-resident, sharded zero buffer for the NEFF "out" input slot.
    # The kernel overwrites every element, so contents are irrelevant and
    # the buffer can be reused across calls (never donated).
    global _ZEROS
    if _ZEROS is None:
        sharding = NamedSharding(mesh, PartitionSpec("core"))
        _ZEROS = jax.jit(
            lambda: jnp.zeros((PAIRS, 128, T), jnp.float32),
            out_shardings=sharding,
        )()
    return _ZEROS


def kernel(x: np.ndarray, weights: np.ndarray, cond_ids: np.ndarray) -> np.ndarray:
    x = np.ascontiguousarray(np.asarray(x, dtype=np.float32))
    weights = np.asarray(weights, dtype=np.float32)
    cond_ids = np.asarray(cond_ids, dtype=np.int32)

    # Host-side routing: gather per-sample weights, pack sample pairs into
    # block-diagonal [128, 128] stationary matrices.
    w_full = weights[cond_ids]                      # [B, CIN, COUT]
    wp = np.zeros((PAIRS, 2 * CIN, 2 * COUT), dtype=np.float32)
    wp[:, :CIN, :COUT] = w_full[0::2]
    wp[:, CIN:, COUT:] = w_full[1::2]

    x_pairs = x.reshape(PAIRS, 2 * CIN, T)          # zero-copy view

    fn, mesh = make_runner(reps=1)
    out = fn(x_pairs, wp, _get_zeros(mesh))
    return np.asarray(out).reshape(B, COUT, T)
